# revision 1
# baseline (speedup 1.0000x reference)
"""Trainium2 Bass kernel for the CherryAllocation NAGNN (grid GIN + MLP head).

Self-contained: hardcodes shapes/sharding. Data-parallel over batch:
64 samples -> 8 NeuronCores x 8 samples. Weights replicated.

Math per sample (grid 32x32, N=1024 nodes):
  mask = obs[:1024] != 0 ; x = obs[1024:].reshape(1024, 32)
  h0 = x
  for l in 0..3:  agg = sum of 4-neighbor h ; h = relu(LN(agg @ Wl + bl) * g + be)
  xc = concat([x, h1, h2, h3, h4])  # [1024, 1056]
  z  = relu(BN(xc @ W1 + b1))       # BN eval-mode affine
  y  = z @ W2 + b2 ; out = where(mask, y, -1e7)

Implementation notes:
 - activations feature-major (FM) [feat, tok]; grid aggregation fused into
   the matmul PSUM accumulation: vertical +-32-token shifts via shifted
   stationary-operand slices over zero guard bands; horizontal +-1 neighbors
   pre-summed on GpSimd (hh).
 - act-stationary matmuls give node-major z blocks [128 tok, 256]; LN stats
   per block-pair (bn_stats/bn_aggr + sqrt + reciprocal), normalize via
   tensor_scalar, PE-transpose back to FM, ACT applies gamma/beta + relu.
 - samples processed in interleaved pairs so one sample's matmuls cover the
   other's LayerNorm chain (keeps the PE warm).
 - matmul operand dtype: float32r (full PE rate at moving dim >= 256) or
   bfloat16 (enables fast weight load), selected by USE_BF16.
"""

import numpy as np

import concourse.bass as bass
import concourse.bacc as bacc
import concourse.mybir as mybir
import concourse.tile as tile
from concourse.bass_utils import run_bass_kernel_spmd
from concourse.masks import make_identity

FP = mybir.dt.float32
FR = mybir.dt.float32r
BF = mybir.dt.bfloat16
AF = mybir.ActivationFunctionType
OP = mybir.AluOpType

GRID = 32
NN = 1024            # nodes per sample
F_IN = 32
H = 256
B = 64
S = 8                # samples per core
NCORE = 8
NB = 8               # 128-token blocks per sample
OBS_W = NN + NN * F_IN   # 33792
MIN_VAL = -10000000.0
EPS_LN = 1e-5
EPS_BN = 1e-5
PAD = 32             # token guard band for vertical shifts
HW = NN + 2 * PAD    # 1088, padded token width per feature-half

USE_BF16 = False
PROFILE = False
LAST_EXEC_NS = None
TRACE_KWARGS = {}


def _build(has_gin_bias: bool, b2_val: float, use_bf16: bool) -> bass.Bass:
    nc = bacc.Bacc("TRN2", target_bir_lowering=False, debug=False)

    MT = BF if use_bf16 else FP          # storage dtype of matmul operands
    GI = mybir.dt.uint16 if use_bf16 else mybir.dt.uint32

    def mm(ap):
        """View an operand/producer AP in the matmul dtype."""
        return ap if use_bf16 else ap.bitcast(FR)

    obs = nc.declare_dram_parameter("obs", [S, OBS_W], FP, isOutput=False)
    w0 = nc.declare_dram_parameter("w0", [F_IN, H], FP, isOutput=False)
    ws = nc.declare_dram_parameter("ws", [3, 2, 128, H], FP, isOutput=False)
    w1x = nc.declare_dram_parameter("w1x", [F_IN, 512], FP, isOutput=False)
    w1h = nc.declare_dram_parameter("w1h", [8, 128, 512], FP, isOutput=False)
    w2 = nc.declare_dram_parameter("w2", [4, 128], FP, isOutput=False)
    gg = nc.declare_dram_parameter("gg", [4, H], FP, isOutput=False)
    bb = nc.declare_dram_parameter("bb", [4, H], FP, isOutput=False)
    bns = nc.declare_dram_parameter("bns", [512], FP, isOutput=False)
    bnt = nc.declare_dram_parameter("bnt", [512], FP, isOutput=False)
    if has_gin_bias:
        gbias = nc.declare_dram_parameter("gbias", [4, H], FP, isOutput=False)
    y_out = nc.declare_dram_parameter("y", [S, NN], FP, isOutput=True)

    from contextlib import ExitStack

    with tile.TileContext(nc) as tc, ExitStack() as ctx:
        wp = ctx.enter_context(tc.tile_pool(name="w", bufs=1))
        px = ctx.enter_context(tc.tile_pool(name="px", bufs=2))
        ph = ctx.enter_context(tc.tile_pool(name="ph", bufs=2))
        pst = ctx.enter_context(tc.tile_pool(name="pst", bufs=8))
        pfin = ctx.enter_context(tc.tile_pool(name="pfin", bufs=1))
        pz = ctx.enter_context(tc.tile_pool(name="pz", bufs=3, space="PSUM"))
        ptf = ctx.enter_context(tc.tile_pool(name="ptf", bufs=5, space="PSUM"))

        # ---- constants / weights in SBUF ----
        ident = wp.tile([128, 128], MT, tag="id")
        make_identity(nc, ident[:])
        eps_sb = wp.tile([128, 1], FP, tag="eps")
        nc.gpsimd.memset(eps_sb[:], EPS_LN)

        w0_sb = wp.tile([F_IN, H], MT, tag="w0")
        nc.gpsimd.dma_start(mm(w0_sb[:]), w0[:, :])

        wl_sb = []
        for l in range(3):
            t = wp.tile([128, 2 * H], MT, tag=f"wl{l}")
            nc.gpsimd.dma_start(
                mm(t[:]).rearrange("p (k n) -> p k n", k=2),
                ws[l].rearrange("k p n -> p k n"),
            )
            wl_sb.append(t)

        w1x_sb = wp.tile([F_IN, 512], MT, tag="w1x")
        nc.gpsimd.dma_start(mm(w1x_sb[:]), w1x[:, :])
        w1h_sb = wp.tile([128, 8 * 512], MT, tag="w1h")
        nc.gpsimd.dma_start(
            mm(w1h_sb[:]).rearrange("p (j m) -> p j m", j=8),
            w1h[:, :, :].rearrange("j p m -> p j m"),
        )
        w2_sb = wp.tile([128, 4], MT, tag="w2")
        nc.gpsimd.dma_start(mm(w2_sb[:]), w2[:, :].rearrange("k p -> p k"))

        gg_sb = wp.tile([128, 8], FP, tag="gg")
        nc.sync.dma_start(
            gg_sb[:].rearrange("p (l c) -> p l c", c=2),
            gg[:, :].rearrange("l (c p) -> p l c", p=128),
        )
        bb_sb = wp.tile([128, 8], FP, tag="bb")
        nc.sync.dma_start(
            bb_sb[:].rearrange("p (l c) -> p l c", c=2),
            bb[:, :].rearrange("l (c p) -> p l c", p=128),
        )
        bns_sb = wp.tile([128, 4], FP, tag="bns")
        nc.sync.dma_start(bns_sb[:], bns[:].rearrange("(m p) -> p m", p=128))
        bnt_sb = wp.tile([128, 4], FP, tag="bnt")
        nc.sync.dma_start(bnt_sb[:], bnt[:].rearrange("(m p) -> p m", p=128))

        if has_gin_bias:
            ones1 = wp.tile([1, 128], MT, tag="ones1")
            if use_bf16:
                nc.gpsimd.memset(ones1[:].bitcast(mybir.dt.uint16), 0x3F80)
            else:
                nc.gpsimd.memset(ones1[:].bitcast(mybir.dt.uint32), 0x3F800000)
            gb_sb = wp.tile([1, 4 * H], MT, tag="gb")
            nc.gpsimd.dma_start(
                mm(gb_sb[:]).rearrange("q (l n) -> q l n", l=4), gbias[:, :]
            )

        def build_hh(hh_tile, src_tile, n_kc, pad):
            """hh[f, t] = h[f, left(t)] + h[f, right(t)]; on GpSimd."""
            for kc in range(n_kc):
                src = src_tile[:, kc * (NN + 2 * pad) + pad
                               : kc * (NN + 2 * pad) + pad + NN]
                dst = hh_tile[:, kc * NN : (kc + 1) * NN]
                sv = src.rearrange("p (r c) -> p r c", c=GRID)
                dv = dst.rearrange("p (r c) -> p r c", c=GRID)
                nc.gpsimd.tensor_add(
                    mm(dv[:, :, 1:31]), sv[:, :, 0:30], sv[:, :, 2:32]
                )
                nc.gpsimd.tensor_copy(mm(dv[:, :, 0:1]), sv[:, :, 1:2])
                nc.gpsimd.tensor_copy(mm(dv[:, :, 31:32]), sv[:, :, 30:31])

        def emit_layer_mms(z, b, lhs_tile, hh_tile, n_kc, rhs_of_kc, l):
            """Fused aggregation matmul group for one token block."""
            mms = []
            for kc in range(n_kc):
                mms.append(
                    (hh_tile[:, kc * NN + b * 128 : kc * NN + b * 128 + 128],
                     rhs_of_kc(kc))
                )
            for kc in range(n_kc):  # up neighbors (t-32)
                base = kc * HW + b * 128
                mms.append((lhs_tile[:, base : base + 128], rhs_of_kc(kc)))
            for kc in range(n_kc):  # down neighbors (t+32)
                base = kc * HW + b * 128 + 64
                mms.append((lhs_tile[:, base : base + 128], rhs_of_kc(kc)))
            n = len(mms) + (1 if has_gin_bias else 0)
            for i, (lhsT, rhs) in enumerate(mms):
                nc.tensor.matmul(
                    z[:, :], mm(lhsT), mm(rhs),
                    start=(i == 0), stop=(i == n - 1),
                )
            if has_gin_bias:
                nc.tensor.matmul(
                    z[:, :], mm(ones1[0:1, 0:128]),
                    mm(gb_sb[0:1, l * H : (l + 1) * H]),
                    start=False, stop=True,
                )

        def prep_x(s):
            """Load x for sample s, transpose to FM with guard bands."""
            x_nm = px.tile([128, 256], MT, tag="xnm")
            dma = nc.gpsimd.dma_start if use_bf16 else nc.sync.dma_start
            dma(
                x_nm[:].rearrange("p (b f) -> p b f", f=F_IN),
                obs[s, NN:OBS_W].rearrange("(b p f) -> p b f", p=128, f=F_IN),
            )
            x_fm = px.tile([F_IN, HW], MT, tag="xfm")
            nc.gpsimd.memset(x_fm[:, 0:PAD].bitcast(GI), 0)
            nc.gpsimd.memset(x_fm[:, PAD + NN : HW].bitcast(GI), 0)
            for half in range(2):
                x_tfm = ptf.tile([F_IN, 512], MT, tag="tf")
                for i in range(4):
                    b = half * 4 + i
                    nc.tensor.transpose(
                        x_tfm[:, i * 128 : (i + 1) * 128],
                        x_nm[:, b * F_IN : (b + 1) * F_IN],
                        ident[:],
                    )
                nc.scalar.copy(
                    mm(x_fm[:, PAD + half * 512 : PAD + (half + 1) * 512]),
                    x_tfm[:],
                )
            hh_x = px.tile([F_IN, NN], MT, tag="hhx")
            build_hh(hh_x, x_fm, 1, PAD)
            return {"s": s, "x_fm": x_fm, "hh_x": hh_x, "h": []}

        def layer_mm_phase(st, l):
            if l == 0:
                n_kc = 1
                prev, prev_hh = st["x_fm"], st["hh_x"]
                rhs_of_kc = lambda kc: w0_sb[:, :]
            else:
                n_kc = 2
                prev, prev_hh = st["h"][l - 1], st["hh"]
                wl = wl_sb[l - 1]
                rhs_of_kc = lambda kc, wl=wl: wl[:, kc * H : (kc + 1) * H]

            t_nm = ph.tile([128, NB * H], MT, tag="tnm")
            for bp in range(4):
                zs = []
                for b in (2 * bp, 2 * bp + 1):
                    z = pz.tile([128, H], FP, tag="z")
                    emit_layer_mms(z, b, prev, prev_hh, n_kc, rhs_of_kc, l)
                    zs.append(z)
                mvp = pst.tile([128, 4], FP, tag="mv")
                for i, z in enumerate(zs):
                    st6 = pst.tile([128, 6], FP, tag="st6")
                    nc.vector.bn_stats(st6[:], z[:, :])
                    nc.vector.bn_aggr(mvp[:, 2 * i : 2 * i + 2], st6[:])
                sdp = pst.tile([128, 2], FP, tag="sd")
                var_view = mvp[:].rearrange("p (b t) -> p t b", t=2)[:, 1, :]
                nc.scalar.activation(
                    sdp[:], var_view, AF.Sqrt, bias=eps_sb[:, 0:1], scale=1.0
                )
                invp = pst.tile([128, 2], FP, tag="inv")
                nc.vector.reciprocal(invp[:], sdp[:])
                for i, z in enumerate(zs):
                    b = 2 * bp + i
                    nc.vector.tensor_scalar(
                        out=t_nm[:, b * H : (b + 1) * H],
                        in0=z[:, :],
                        scalar1=mvp[:, 2 * i : 2 * i + 1],
                        scalar2=invp[:, i : i + 1],
                        op0=OP.subtract,
                        op1=OP.mult,
                    )
            st["t_nm"] = t_nm

        def layer_tr_phase(st, l):
            t_nm = st.pop("t_nm")
            h_t = ph.tile([128, 2 * HW], MT, tag=f"h{l}")
            nc.gpsimd.memset(h_t[:, 0:PAD].bitcast(GI), 0)
            nc.gpsimd.memset(h_t[:, PAD + NN : HW + PAD].bitcast(GI), 0)
            nc.gpsimd.memset(h_t[:, HW + PAD + NN : 2 * HW].bitcast(GI), 0)
            for half in range(2):
                for c in range(2):
                    tf = ptf.tile([128, 512], MT, tag="tf", name="tfc")
                    for i in range(4):
                        b = half * 4 + i
                        nc.tensor.transpose(
                            tf[:, i * 128 : (i + 1) * 128],
                            t_nm[:, b * H + c * 128 : b * H + c * 128 + 128],
                            ident[:],
                        )
                    nc.scalar.activation(
                        mm(h_t[:, c * HW + PAD + half * 512
                               : c * HW + PAD + (half + 1) * 512]),
                        tf[:],
                        AF.Relu,
                        scale=gg_sb[:, l * 2 + c : l * 2 + c + 1],
                        bias=bb_sb[:, l * 2 + c : l * 2 + c + 1],
                    )
            st["h"].append(h_t)
            if l < 3:
                hh_t = ph.tile([128, 2 * NN], MT, tag="hh")
                build_hh(hh_t, h_t, 2, PAD)
                st["hh"] = hh_t

        def unit_w1(st):
            z_sb = ph.tile([128, 4096], MT, tag="zsb")
            for m in range(4):
                for c2 in range(2):
                    zw1 = ptf.tile([128, 512], FP, tag="tf")
                    for kc in range(9):
                        if kc == 0:
                            lhsT = w1x_sb[:, m * 128 : (m + 1) * 128]
                            rt, roff = st["x_fm"], 0
                        else:
                            j = kc - 1
                            lhsT = w1h_sb[:, j * 512 + m * 128
                                          : j * 512 + (m + 1) * 128]
                            rt, roff = st["h"][j // 2], (j % 2) * HW
                        nc.tensor.matmul(
                            zw1[:, :],
                            mm(lhsT),
                            mm(rt[:, roff + PAD + c2 * 512
                                   : roff + PAD + (c2 + 1) * 512]),
                            start=(kc == 0), stop=(kc == 8),
                        )
                    nc.scalar.activation(
                        mm(z_sb[:, m * NN + c2 * 512 : m * NN + (c2 + 1) * 512]),
                        zw1[:],
                        AF.Relu,
                        scale=bns_sb[:, m : m + 1],
                        bias=bnt_sb[:, m : m + 1],
                    )
            st["z_sb"] = z_sb

        def unit_w2(st):
            s = st["s"]
            z_sb = st["z_sb"]
            y_s = pfin.tile([1, NN], FP, tag="ys", bufs=2)
            for c2 in range(2):
                yp = pz.tile([1, 512], FP, tag="z")
                for m in range(4):
                    nc.tensor.matmul(
                        yp[0:1, :],
                        mm(w2_sb[:, m : m + 1]),
                        mm(z_sb[:, m * NN + c2 * 512 : m * NN + (c2 + 1) * 512]),
                        start=(m == 0), stop=(m == 3),
                    )
                nc.vector.tensor_copy(y_s[:, c2 * 512 : (c2 + 1) * 512], yp[0:1, :])
            if b2_val != 0.0:
                nc.scalar.add(y_s[:], y_s[:], b2_val)
            m_s = pfin.tile([1, NN], FP, tag="ms", bufs=2)
            nc.sync.dma_start(m_s[:], obs[s : s + 1, 0:NN])
            yf = pfin.tile([1, NN], FP, tag="yfin", bufs=2)
            nc.gpsimd.memset(yf[:], MIN_VAL)
            nc.vector.copy_predicated(yf[:], m_s[:].bitcast(mybir.dt.uint32), y_s[:])
            nc.sync.dma_start(y_out[s : s + 1, :], yf[:])

        # ---- interleaved sample pairs: partner matmuls hide LN latency.
        # Both samples' matmul phases are emitted before either sample's
        # transpose phase so the PE instruction stream never waits on the
        # just-issued LayerNorm chain.
        for p in range(S // 2):
            sts = [prep_x(2 * p), prep_x(2 * p + 1)]
            for l in range(4):
                for st in sts:
                    layer_mm_phase(st, l)
                for st in sts:
                    layer_tr_phase(st, l)
            for st in sts:
                unit_w1(st)
            for st in sts:
                unit_w2(st)

    nc.finalize()
    return nc


_BUILD_CACHE = {}


def _get_nc(has_gin_bias: bool, b2_val: float, use_bf16: bool) -> bass.Bass:
    key = (has_gin_bias, float(b2_val), use_bf16)
    if key not in _BUILD_CACHE:
        _BUILD_CACHE[key] = _build(has_gin_bias, b2_val, use_bf16)
    return _BUILD_CACHE[key]


def prep_maps(observations, W0, b0, g0, be0, Ws, bs, gs, bes,
              W1, b1, bn_g, bn_b, bn_m, bn_v, W2, b2, **_ignored):
    obs = np.ascontiguousarray(np.asarray(observations, np.float32))
    W0 = np.ascontiguousarray(np.asarray(W0, np.float32))
    Ws = np.asarray(Ws, np.float32)
    W1 = np.asarray(W1, np.float32)
    W2 = np.asarray(W2, np.float32)
    gg = np.ascontiguousarray(np.stack(
        [np.asarray(g0, np.float32)] + [np.asarray(gs, np.float32)[i] for i in range(3)]))
    bb = np.ascontiguousarray(np.stack(
        [np.asarray(be0, np.float32)] + [np.asarray(bes, np.float32)[i] for i in range(3)]))
    gbias = np.ascontiguousarray(np.stack(
        [np.asarray(b0, np.float32)] + [np.asarray(bs, np.float32)[i] for i in range(3)]))
    has_gin_bias = bool(np.any(gbias != 0.0))
    bn_scale = (np.asarray(bn_g, np.float32)
                / np.sqrt(np.asarray(bn_v, np.float32) + EPS_BN)).astype(np.float32)
    bn_shift = ((np.asarray(b1, np.float32) - np.asarray(bn_m, np.float32)) * bn_scale
                + np.asarray(bn_b, np.float32)).astype(np.float32)
    b2_val = float(np.asarray(b2, np.float32).reshape(-1)[0])

    ws_r = np.ascontiguousarray(Ws.reshape(3, 2, 128, H))
    w1x = np.ascontiguousarray(W1[:F_IN])
    w1h = np.ascontiguousarray(W1[F_IN:].reshape(8, 128, 512))
    w2r = np.ascontiguousarray(W2.reshape(4, 128))

    shared = {
        "w0": W0, "ws": ws_r, "w1x": w1x, "w1h": w1h, "w2": w2r,
        "gg": gg, "bb": bb, "bns": bn_scale, "bnt": bn_shift,
    }
    if has_gin_bias:
        shared["gbias"] = gbias
    in_maps = []
    for c in range(NCORE):
        m = dict(shared)
        m["obs"] = np.ascontiguousarray(obs[c * S : (c + 1) * S])
        in_maps.append(m)
    return in_maps, has_gin_bias, b2_val


def kernel(**inputs) -> np.ndarray:
    global LAST_EXEC_NS
    in_maps, has_gin_bias, b2_val = prep_maps(**inputs)
    nc = _get_nc(has_gin_bias, b2_val, USE_BF16)
    res = run_bass_kernel_spmd(
        nc, in_maps, list(range(NCORE)), trace=PROFILE, **TRACE_KWARGS
    )
    LAST_EXEC_NS = res.exec_time_ns
    y = np.concatenate([res.results[c]["y"] for c in range(NCORE)], axis=0)
    return y.reshape(B, NN).astype(np.float32)



# revision 2
# speedup vs baseline: 1.0194x; 1.0194x over previous
"""Trainium2 Bass kernel for the CherryAllocation NAGNN (grid GIN + MLP head).

v3: presummed aggregation + pair/cross-pair pipelining + bf16 operands.

Self-contained: hardcodes shapes/sharding. Data-parallel over batch:
64 samples -> 8 NeuronCores x 8 samples. Weights replicated.

Math per sample (grid 32x32, N=1024 nodes):
  mask = obs[:1024] != 0 ; x = obs[1024:].reshape(1024, 32)
  h0 = x
  for l in 0..3:  agg = sum of 4-neighbor h ; h = relu(LN(agg @ Wl + bl) * g + be)
  xc = concat([x, h1, h2, h3, h4])  # [1024, 1056]
  z  = relu(BN(xc @ W1 + b1))       # BN eval-mode affine
  y  = z @ W2 + b2 ; out = where(mask, y, -1e7)

Implementation notes:
 - activations feature-major (FM) [feat, tok]; the 4-neighbor aggregation is
   fully pre-summed off the PE: vv = up+down (+-32 shifts) on Vector,
   hh = left+right as full-width +-1 contiguous shifted adds on GpSimd,
   agg = hh+vv on GpSimd, then two small Vector fixup-adds repair the grid-row
   boundary columns (where the +-1 shift crossed a row). Each layer block then
   needs only n_kc matmuls (vs 3*n_kc with matmul-fused shifts).
 - LN per block-pair: z pair in one [128,512] PSUM bank; z copied PSUM->SBUF
   bf16 (alternating Vector/Scalar engines) - this copy is mandatory anyway
   since the PE transpose can't read PSUM; bn_stats/bn_aggr on the bf16 copy;
   sqrt/recip batched per layer [128,8]; normalize via tensor_scalar (sub
   mean, mul inv) into bf16 t_nm.
 - t_nm PE-transposed back to FM; relu(*gamma+beta) applied during the
   PSUM->SBUF copy (Scalar engine; Vector max0 when gamma=1,beta=0).
 - two-level pipelining: samples processed in interleaved pairs (A,B), and
   pair p's vector-heavy GIN layers overlap pair p-1's PE-heavy W1/W2
   matmuls (4 W1 chunks interleaved per layer), keeping the PE dense enough
   to hold the HAM clock at 2.4 GHz.
"""

import numpy as np

import concourse.bass as bass
import concourse.bacc as bacc
import concourse.mybir as mybir
import concourse.tile as tile
from concourse.bass_utils import run_bass_kernel_spmd
from concourse.masks import make_identity

FP = mybir.dt.float32
FR = mybir.dt.float32r
BF = mybir.dt.bfloat16
AF = mybir.ActivationFunctionType
OP = mybir.AluOpType

GRID = 32
NN = 1024            # nodes per sample
F_IN = 32
H = 256
B = 64
S = 8                # samples per core
NCORE = 8
NB = 8               # 128-token blocks per sample
OBS_W = NN + NN * F_IN   # 33792
MIN_VAL = -10000000.0
EPS_LN = 1e-5
EPS_BN = 1e-5
PAD = 32             # token guard band for +-32 vertical shifts
HW = NN + 2 * PAD    # 1088, padded token width per feature-half

USE_BF16 = True
PROFILE = False
LAST_EXEC_NS = None
TRACE_KWARGS = {}


def _build(has_gin_bias: bool, ln_trivial: bool, b2_val: float,
           use_bf16: bool) -> bass.Bass:
    nc = bacc.Bacc("TRN2", target_bir_lowering=False, debug=False)

    MT = BF if use_bf16 else FP          # storage dtype of matmul operands

    def mm(ap):
        """View an operand/producer AP in the matmul dtype."""
        return ap if use_bf16 else ap.bitcast(FR)

    obs = nc.declare_dram_parameter("obs", [S, OBS_W], FP, isOutput=False)
    w0 = nc.declare_dram_parameter("w0", [F_IN, H], FP, isOutput=False)
    ws = nc.declare_dram_parameter("ws", [3, 2, 128, H], FP, isOutput=False)
    w1x = nc.declare_dram_parameter("w1x", [F_IN, 512], FP, isOutput=False)
    w1h = nc.declare_dram_parameter("w1h", [8, 128, 512], FP, isOutput=False)
    w2 = nc.declare_dram_parameter("w2", [4, 128], FP, isOutput=False)
    gg = nc.declare_dram_parameter("gg", [4, H], FP, isOutput=False)
    bb = nc.declare_dram_parameter("bb", [4, H], FP, isOutput=False)
    bns = nc.declare_dram_parameter("bns", [512], FP, isOutput=False)
    bnt = nc.declare_dram_parameter("bnt", [512], FP, isOutput=False)
    if has_gin_bias:
        gba = nc.declare_dram_parameter("gba", [4, H], FP, isOutput=False)
    y_out = nc.declare_dram_parameter("y", [S, NN], FP, isOutput=True)

    from contextlib import ExitStack

    with tile.TileContext(nc) as tc, ExitStack() as ctx:
        wp = ctx.enter_context(tc.tile_pool(name="w", bufs=1))
        px = ctx.enter_context(tc.tile_pool(name="px", bufs=4))
        ph = ctx.enter_context(tc.tile_pool(name="ph", bufs=2))
        pst = ctx.enter_context(tc.tile_pool(name="pst", bufs=2))
        pfin = ctx.enter_context(tc.tile_pool(name="pfin", bufs=2))
        pz = ctx.enter_context(tc.tile_pool(name="pz", bufs=4, space="PSUM"))
        ptf = ctx.enter_context(tc.tile_pool(name="ptf", bufs=2, space="PSUM"))
        pw = ctx.enter_context(tc.tile_pool(name="pw", bufs=2, space="PSUM"))

        # ---- constants / weights in SBUF ----
        ident = wp.tile([128, 128], MT, tag="id")
        make_identity(nc, ident[:])
        eps_sb = wp.tile([128, 1], FP, tag="eps")
        nc.gpsimd.memset(eps_sb[:], EPS_LN)

        w0_sb = wp.tile([F_IN, H], MT, tag="w0")
        nc.gpsimd.dma_start(mm(w0_sb[:]), w0[:, :])

        wl_sb = []
        for l in range(3):
            t = wp.tile([128, 2 * H], MT, tag=f"wl{l}")
            nc.gpsimd.dma_start(
                mm(t[:]).rearrange("p (k n) -> p k n", k=2),
                ws[l].rearrange("k p n -> p k n"),
            )
            wl_sb.append(t)

        w1x_sb = wp.tile([F_IN, 512], MT, tag="w1x")
        nc.gpsimd.dma_start(mm(w1x_sb[:]), w1x[:, :])
        w1h_sb = wp.tile([128, 8 * 512], MT, tag="w1h")
        nc.gpsimd.dma_start(
            mm(w1h_sb[:]).rearrange("p (j m) -> p j m", j=8),
            w1h[:, :, :].rearrange("j p m -> p j m"),
        )
        w2_sb = wp.tile([128, 4], MT, tag="w2")
        nc.gpsimd.dma_start(mm(w2_sb[:]), w2[:, :].rearrange("k p -> p k"))

        gg_sb = wp.tile([128, 8], FP, tag="gg")
        nc.sync.dma_start(
            gg_sb[:].rearrange("p (l c) -> p l c", c=2),
            gg[:, :].rearrange("l (c p) -> p l c", p=128),
        )
        bb_sb = wp.tile([128, 8], FP, tag="bb")
        nc.sync.dma_start(
            bb_sb[:].rearrange("p (l c) -> p l c", c=2),
            bb[:, :].rearrange("l (c p) -> p l c", p=128),
        )
        bns_sb = wp.tile([128, 4], FP, tag="bns")
        nc.sync.dma_start(bns_sb[:], bns[:].rearrange("(m p) -> p m", p=128))
        bnt_sb = wp.tile([128, 4], FP, tag="bnt")
        nc.sync.dma_start(bnt_sb[:], bnt[:].rearrange("(m p) -> p m", p=128))

        if has_gin_bias:
            ones1 = wp.tile([1, 128], MT, tag="ones1")
            if use_bf16:
                nc.gpsimd.memset(ones1[:].bitcast(mybir.dt.uint16), 0x3F80)
            else:
                nc.gpsimd.memset(ones1[:].bitcast(mybir.dt.uint32), 0x3F800000)
            gb_sb = wp.tile([1, 4 * H], MT, tag="gb")
            nc.gpsimd.dma_start(
                mm(gb_sb[:]).rearrange("q (l n) -> q l n", l=4), gba[:, :]
            )

        # round-robin over PSUM->SBUF copy engines for load balance
        eng_ctr = [0]

        def copy_alt(dst, src):
            eng_ctr[0] ^= 1
            if eng_ctr[0]:
                nc.vector.tensor_copy(dst, src)
            else:
                nc.scalar.copy(dst, src)

        def fm_memset(t, n_kc):
            """Zero the guard bands of an FM tile."""
            for kc in range(n_kc):
                nc.gpsimd.memset(t[:, kc * HW : kc * HW + PAD], 0.0)
                nc.gpsimd.memset(t[:, (kc + 1) * HW - PAD : (kc + 1) * HW],
                                 0.0)

        def build_agg(src_tile, n_kc, np_, tag):
            """Full 4-neighbor presum of src (FM, guard-banded) -> agg tile
            (same guarded layout, tokens at kc*HW+PAD).

            vv (+-32) and the combine are flat full-width shifted adds; the
            +-1 horizontal add is also flat, which wrongly includes the
            neighboring grid row's edge value at the 2 boundary columns of
            each 32-col grid row - small strided GpSimd adds then overwrite
            those 64 columns per chunk with the correct value."""
            W = n_kc * HW
            sv = src_tile
            hh = ph.tile([np_, W], MT, tag=f"hh{tag}")
            vv = ph.tile([np_, W], MT, tag=f"vv{tag}")
            agg = ph.tile([np_, W], MT, tag=f"agg{tag}")
            s4 = sv[:].rearrange("p (k w) -> p k w", k=n_kc)[:, :, PAD : PAD + NN]
            s4 = s4.rearrange("p k (r c) -> p k r c", c=GRID)
            v4 = vv[:].rearrange("p (k w) -> p k w", k=n_kc)[:, :, PAD : PAD + NN]
            v4 = v4.rearrange("p k (r c) -> p k r c", c=GRID)
            a4 = agg[:].rearrange("p (k w) -> p k w", k=n_kc)[:, :, PAD : PAD + NN]
            a4 = a4.rearrange("p k (r c) -> p k r c", c=GRID)
            # two token-range parts so early blocks' matmuls unblock sooner
            # (part A = grid rows 0-7; its +-1/+-32 halo stays within the
            # first relu-copy half, rows 0-15)
            for (t0, t1, r0, r1) in ((0, 256, 0, 8), (256, NN, 8, 32)):
                for kc in range(n_kc):
                    b0 = kc * HW + PAD + t0
                    b1 = kc * HW + PAD + t1
                    nc.vector.tensor_add(hh[:, b0:b1], sv[:, b0 - 1 : b1 - 1],
                                         sv[:, b0 + 1 : b1 + 1])
                    nc.vector.tensor_add(vv[:, b0:b1],
                                         sv[:, b0 - PAD : b1 - PAD],
                                         sv[:, b0 + PAD : b1 + PAD])
                    if kc == 0:
                        nc.vector.tensor_add(agg[:, b0:b1], hh[:, b0:b1],
                                             vv[:, b0:b1])
                    else:
                        nc.gpsimd.tensor_add(agg[:, b0:b1], hh[:, b0:b1],
                                             vv[:, b0:b1])
                nc.gpsimd.tensor_add(
                    a4[:, :, r0:r1, 0:1], s4[:, :, r0:r1, 1:2],
                    v4[:, :, r0:r1, 0:1]
                )
                nc.gpsimd.tensor_add(
                    a4[:, :, r0:r1, 31:32], s4[:, :, r0:r1, 30:31],
                    v4[:, :, r0:r1, 31:32]
                )
            return agg

        def prep_x(s):
            """Load x for sample s, transpose to FM with guard bands."""
            x_nm = px.tile([128, 256], MT, tag="xnm", bufs=2)
            dma = nc.gpsimd.dma_start if use_bf16 else nc.sync.dma_start
            dma(
                x_nm[:].rearrange("p (b f) -> p b f", f=F_IN),
                obs[s, NN:OBS_W].rearrange("(b p f) -> p b f", p=128, f=F_IN),
            )
            x_fm = px.tile([F_IN, HW], MT, tag="xfm")
            fm_memset(x_fm, 1)
            for half in range(2):
                x_tfm = ptf.tile([128, 512], MT, tag="tf")
                for i in range(4):
                    b = half * 4 + i
                    nc.tensor.transpose(
                        x_tfm[0:F_IN, i * 128 : (i + 1) * 128],
                        x_nm[:, b * F_IN : (b + 1) * F_IN],
                        ident[:],
                    )
                nc.scalar.copy(
                    x_fm[:, PAD + half * 512 : PAD + (half + 1) * 512],
                    x_tfm[0:F_IN, :],
                )
            st = {"s": s, "x_fm": x_fm, "h": []}
            st["agg"] = build_agg(x_fm, 1, F_IN, "x")
            return st

        def layer_mm(st, l):
            """Matmuls + LN stats for one layer of one sample."""
            if l == 0:
                n_kc = 1
                rhs_of_kc = lambda kc: w0_sb[:, :]
            else:
                n_kc = 2
                wl = wl_sb[l - 1]
                rhs_of_kc = lambda kc, wl=wl: wl[:, kc * H : (kc + 1) * H]
            agg = st.pop("agg")

            mv = pst.tile([128, 16], FP, tag="mv")
            zcs = []
            for p in range(4):
                zp = pz.tile([128, 512], FP, tag="z")
                for i, b in ((0, 2 * p), (1, 2 * p + 1)):
                    n_mm = n_kc + (1 if has_gin_bias else 0)
                    for kc in range(n_kc):
                        nc.tensor.matmul(
                            zp[:, i * 256 : (i + 1) * 256],
                            mm(agg[:, kc * HW + PAD + b * 128
                                   : kc * HW + PAD + b * 128 + 128]),
                            mm(rhs_of_kc(kc)),
                            start=(kc == 0), stop=(kc == n_mm - 1),
                        )
                    if has_gin_bias:
                        nc.tensor.matmul(
                            zp[:, i * 256 : (i + 1) * 256],
                            mm(ones1[0:1, 0:128]),
                            mm(gb_sb[0:1, l * H : (l + 1) * H]),
                            start=False, stop=True,
                        )
                zc = ph.tile([128, 512], MT, tag="zc", bufs=8)
                nc.scalar.copy(zc[:], zp[:])
                for i in range(2):
                    b = 2 * p + i
                    st6 = pst.tile([128, 6], FP, tag="st6", bufs=4)
                    nc.vector.bn_stats(st6[:], zp[:, i * 256 : (i + 1) * 256])
                    nc.vector.bn_aggr(mv[:, 2 * b : 2 * b + 2], st6[:])
                zcs.append(zc)
            st["mv"] = mv
            st["zcs"] = zcs

        def layer_norm(st, l):
            """Batched sqrt/recip + per-block normalize into t_nm."""
            mv = st.pop("mv")
            zcs = st.pop("zcs")
            var_view = mv[:].rearrange("p (b t) -> p t b", t=2)[:, 1, :]
            sg = pst.tile([128, 8], FP, tag="sg")
            nc.scalar.activation(sg[:], var_view, AF.Sqrt,
                                 bias=eps_sb[:, 0:1], scale=1.0)
            iv = pst.tile([128, 8], FP, tag="iv")
            nc.vector.reciprocal(iv[:], sg[:])
            nm = pst.tile([128, 8], FP, tag="nm")
            mean_view = mv[:].rearrange("p (b t) -> p t b", t=2)[:, 0, :]
            nc.vector.tensor_tensor(out=nm[:], in0=mean_view, in1=iv[:],
                                    op=OP.mult)
            nc.vector.tensor_scalar(out=nm[:], in0=nm[:], scalar1=-1.0,
                                    scalar2=None, op0=OP.mult)
            t_nm = ph.tile([128, 2048], MT, tag="tnm")
            for b in range(8):
                blk = zcs[b // 2][:, (b % 2) * 256 : (b % 2) * 256 + 256]
                out = t_nm[:, b * 256 : (b + 1) * 256]
                if b % 2 == 0:
                    nc.vector.tensor_scalar(
                        out=out, in0=blk,
                        scalar1=mv[:, 2 * b : 2 * b + 1],
                        scalar2=iv[:, b : b + 1],
                        op0=OP.subtract, op1=OP.mult,
                    )
                else:
                    nc.scalar.activation(
                        out, blk, AF.Identity,
                        scale=iv[:, b : b + 1], bias=nm[:, b : b + 1],
                    )
            st["t_nm"] = t_nm

        def layer_tr(st, l):
            """Transpose normalized blocks back to FM, relu(*g+b) on copy."""
            t_nm = st.pop("t_nm")
            h_t = ph.tile([128, 2 * HW], MT, tag=f"h{l}", bufs=4)
            fm_memset(h_t, 2)
            for c in range(2):
                for half in range(2):
                    tf = ptf.tile([128, 512], MT, tag="tf")
                    for i in range(4):
                        b = half * 4 + i
                        nc.tensor.transpose(
                            tf[:, i * 128 : (i + 1) * 128],
                            t_nm[:, b * 256 + c * 128 : b * 256 + c * 128 + 128],
                            ident[:],
                        )
                    dst = h_t[:, c * HW + PAD + half * 512
                              : c * HW + PAD + (half + 1) * 512]
                    src = tf[:]
                    nc.scalar.activation(
                        dst, src, AF.Relu,
                        scale=gg_sb[:, l * 2 + c : l * 2 + c + 1],
                        bias=bb_sb[:, l * 2 + c : l * 2 + c + 1],
                    )
            st["h"].append(h_t)
            if l < 3:
                st["agg"] = build_agg(h_t, 2, 128, "h")

        def w1_chunk(st, m, c2):
            zw1 = pw.tile([128, 512], FP, tag="w")
            for kc in range(9):
                if kc == 0:
                    lhsT = w1x_sb[:, m * 128 : (m + 1) * 128]
                    rt, roff = st["x_fm"], 0
                else:
                    j = kc - 1
                    lhsT = w1h_sb[:, j * 512 + m * 128
                                  : j * 512 + (m + 1) * 128]
                    rt, roff = st["h"][j // 2], j % 2
                nc.tensor.matmul(
                    zw1[:, :],
                    mm(lhsT),
                    mm(rt[:, roff * HW + PAD + c2 * 512
                           : roff * HW + PAD + (c2 + 1) * 512]),
                    start=(kc == 0), stop=(kc == 8),
                )
            nc.scalar.activation(
                st["z_sb"][:, m * NN + c2 * 512 : m * NN + (c2 + 1) * 512],
                zw1[:],
                AF.Relu,
                scale=bns_sb[:, m : m + 1],
                bias=bnt_sb[:, m : m + 1],
            )

        def w2_final(st):
            s = st["s"]
            z_sb = st["z_sb"]
            y_s = pfin.tile([1, NN], FP, tag="ys")
            for c2 in range(2):
                yp = pw.tile([128, 512], FP, tag="w")
                for m in range(4):
                    nc.tensor.matmul(
                        yp[0:1, :],
                        mm(w2_sb[:, m : m + 1]),
                        mm(z_sb[:, m * NN + c2 * 512 : m * NN + (c2 + 1) * 512]),
                        start=(m == 0), stop=(m == 3),
                    )
                nc.vector.tensor_copy(y_s[:, c2 * 512 : (c2 + 1) * 512],
                                      yp[0:1, :])
            if b2_val != 0.0:
                nc.scalar.add(y_s[:], y_s[:], b2_val)
            m_s = pfin.tile([1, NN], FP, tag="ms")
            nc.sync.dma_start(m_s[:], obs[s : s + 1, 0:NN])
            yf = pfin.tile([1, NN], FP, tag="yfin")
            nc.gpsimd.memset(yf[:], MIN_VAL)
            nc.vector.copy_predicated(yf[:], m_s[:].bitcast(mybir.dt.uint32),
                                      y_s[:])
            nc.sync.dma_start(y_out[s : s + 1, :], yf[:])

        def w_closures(st):
            zsb = ph.tile([128, 4096], MT, tag="zsb")
            st["z_sb"] = zsb
            cls = []
            for m in range(4):
                for c2 in range(2):
                    cls.append(lambda m=m, c2=c2: w1_chunk(st, m, c2))
            return cls

        # ---- pipeline: pair (A,B) GIN layers overlap prev pair's W stage ----
        pending = []

        def pump(n):
            for _ in range(n):
                if pending:
                    pending.pop(0)()

        for pr in range(S // 2):
            pump(1)
            sts = [prep_x(2 * pr), prep_x(2 * pr + 1)]
            pump(1)
            for l in range(4):
                for st in sts:
                    layer_mm(st, l)
                    pump(1)
                for st in sts:
                    layer_norm(st, l)
                pump(1)
                for st in sts:
                    layer_tr(st, l)
                pump(1)
            while pending:
                pending.pop(0)()
            pending = []
            for a, b in zip(*[w_closures(st) for st in sts]):
                pending.extend([a, b])
            pending.append(lambda st=sts[0]: w2_final(st))
            pending.append(lambda st=sts[1]: w2_final(st))
        while pending:
            pending.pop(0)()

    nc.finalize()
    return nc


_BUILD_CACHE = {}


def _get_nc(has_gin_bias: bool, ln_trivial: bool, b2_val: float,
            use_bf16: bool) -> bass.Bass:
    key = (has_gin_bias, ln_trivial, float(b2_val), use_bf16)
    if key not in _BUILD_CACHE:
        _BUILD_CACHE[key] = _build(has_gin_bias, ln_trivial, b2_val, use_bf16)
    return _BUILD_CACHE[key]


def prep_maps(observations, W0, b0, g0, be0, Ws, bs, gs, bes,
              W1, b1, bn_g, bn_b, bn_m, bn_v, W2, b2, **_ignored):
    obs = np.ascontiguousarray(np.asarray(observations, np.float32))
    W0 = np.ascontiguousarray(np.asarray(W0, np.float32))
    Ws = np.asarray(Ws, np.float32)
    W1 = np.asarray(W1, np.float32)
    W2 = np.asarray(W2, np.float32)
    gg = np.ascontiguousarray(np.stack(
        [np.asarray(g0, np.float32)] + [np.asarray(gs, np.float32)[i] for i in range(3)]))
    bb = np.ascontiguousarray(np.stack(
        [np.asarray(be0, np.float32)] + [np.asarray(bes, np.float32)[i] for i in range(3)]))
    ln_trivial = bool(np.all(gg == 1.0) and np.all(bb == 0.0))
    gbias = np.ascontiguousarray(np.stack(
        [np.asarray(b0, np.float32)] + [np.asarray(bs, np.float32)[i] for i in range(3)]))
    has_gin_bias = bool(np.any(gbias != 0.0))
    bn_scale = (np.asarray(bn_g, np.float32)
                / np.sqrt(np.asarray(bn_v, np.float32) + EPS_BN)).astype(np.float32)
    bn_shift = ((np.asarray(b1, np.float32) - np.asarray(bn_m, np.float32)) * bn_scale
                + np.asarray(bn_b, np.float32)).astype(np.float32)
    b2_val = float(np.asarray(b2, np.float32).reshape(-1)[0])

    ws_r = np.ascontiguousarray(Ws.reshape(3, 2, 128, H))
    w1x = np.ascontiguousarray(W1[:F_IN])
    w1h = np.ascontiguousarray(W1[F_IN:].reshape(8, 128, 512))
    w2r = np.ascontiguousarray(W2.reshape(4, 128))

    shared = {
        "w0": W0, "ws": ws_r, "w1x": w1x, "w1h": w1h, "w2": w2r,
        "gg": gg, "bb": bb, "bns": bn_scale, "bnt": bn_shift,
    }
    if has_gin_bias:
        shared["gba"] = gbias
    in_maps = []
    for c in range(NCORE):
        m = dict(shared)
        m["obs"] = np.ascontiguousarray(obs[c * S : (c + 1) * S])
        in_maps.append(m)
    return in_maps, has_gin_bias, ln_trivial, b2_val


def kernel(**inputs) -> np.ndarray:
    global LAST_EXEC_NS
    in_maps, has_gin_bias, ln_trivial, b2_val = prep_maps(**inputs)
    nc = _get_nc(has_gin_bias, ln_trivial, b2_val, USE_BF16)
    res = run_bass_kernel_spmd(
        nc, in_maps, list(range(NCORE)), trace=PROFILE, **TRACE_KWARGS
    )
    LAST_EXEC_NS = res.exec_time_ns
    y = np.concatenate([res.results[c]["y"] for c in range(NCORE)], axis=0)
    return y.reshape(B, NN).astype(np.float32)
